# revision 3
# baseline (speedup 1.0000x reference)
"""Trainium2 Bass kernel for nn_LSTMModel (2-layer LSTM captioner + vocab classifier).

v3 (vs v2):
- gx = x @ U1 precomputed on HOST; DMA'd straight into the G1 PSUM slots
  (2 steps / 64 rows per transfer). Removes the on-device x-side GEMM.
- Output logits written bf16 (host upcasts); wave stores alternate between
  the SP (HWDGE) and GpSimd (SWDGE) queues to spread DMA engines.
- PE loop software-pipelined: W1(t+1) issues right after U2p(t), before the
  h2 transposes of step t, so the L2 elementwise chain hides under matmuls.
- ysT is fed from the same bf16 h2 transpose used for the recurrence
  (single transpose group, 8 PE transposes per step).

Layout: every core runs the full-batch recurrence (replicated — gate matmul
cost streams the weights and is batch-independent); the classifier is
vocab-sharded with the Wcy slice resident in SBUF, interleaved 2 chunks per
step into the recurrence.

PSUM (7 of 8 banks): G1 [128,2048] (4) | trp (1) | cps x2 (2)
  G1 partitions 0-31 / 32-63: g1 slots for even/odd steps (gx-DMA + W1);
  G1 partitions 64-95: g2 (free-dim chunked);  96-127 unused.
"""
import sys

sys.path.insert(0, "/opt/trn_rl_repo")
import numpy as np

B, S, L, H, D, V, F = 32, 128, 2, 512, 512, 32000, 768
NCORES = 8
T = S + 1
TB = T * B                 # 4128 gx rows
KT = H // 128
G4 = 4 * H
VSH = V // NCORES          # 4000
VCH = 500
NCH = VSH // VCH           # 8
NWAVE = S // 4             # 32
NGXW = (TB + 127) // 128   # 33 gx ring transfers (last is 32 rows)

TRACE = False

NDMA_B = 3 * KT + 2 * KT + 3   # W1/W2/U2 + h0T x2*KT + c0 x2 + idr
NDMA_C = KT


def _build(nc, bass, mybir, ctx):
    f32 = mybir.dt.float32
    bf16 = mybir.dt.bfloat16
    AF = mybir.ActivationFunctionType

    # ---- DRAM I/O ----
    gx_d = nc.declare_dram_parameter("gx", [TB, G4], bf16, isOutput=False)
    W1_d = nc.declare_dram_parameter("W1", [KT, 128, G4], bf16, isOutput=False)
    W2_d = nc.declare_dram_parameter("W2", [KT, 128, G4], bf16, isOutput=False)
    U2_d = nc.declare_dram_parameter("U2", [KT, 128, G4], bf16, isOutput=False)
    Wc_d = nc.declare_dram_parameter("Wcy", [KT, 128, VSH], bf16, isOutput=False)
    h0_d = nc.declare_dram_parameter("h0T", [L, KT, 128, B], bf16, isOutput=False)
    c0_d = nc.declare_dram_parameter("c0r", [L, B, H], f32, isOutput=False)
    idr_d = nc.declare_dram_parameter("idr", [128, B], bf16, isOutput=False)
    out_d = nc.declare_dram_parameter("out", [B, S, VSH], bf16, isOutput=True)

    # ---- SBUF ----
    sb = lambda name, shape, dt: ctx.enter_context(nc.sbuf_tensor(name, shape, dt))
    W1 = sb("W1s", [128, KT, G4], bf16)
    W2 = sb("W2s", [128, KT, G4], bf16)
    U2 = sb("U2s", [128, KT, G4], bf16)
    Wc = sb("Wcs", [128, KT, VSH], bf16)
    hT1 = sb("hT1s", [128, KT, B], bf16)
    hT2 = sb("hT2s", [128, KT, B], bf16)
    ysT = sb("ysTs", [128, KT, 2, 128], bf16)
    c1 = sb("c1s", [B, H], f32)
    c2 = sb("c2s", [96, H], f32)
    sfio1 = sb("sfio1s", [B, 3 * H], f32)
    tcc1 = sb("tcc1s", [B, H], f32)
    tch1 = sb("tch1s", [B, H], f32)
    tm1 = sb("tm1s", [B, H], f32)
    tm2 = sb("tm2s", [B, H], f32)
    h1r = sb("h1rs", [B, H], bf16)
    sfio2 = sb("sfio2s", [96, 3 * H], f32)
    tcc2 = sb("tcc2s", [96, H], f32)
    tch2 = sb("tch2s", [96, H], f32)
    tm3 = sb("tm3s", [96, H], f32)
    tm4 = sb("tm4s", [96, H], f32)
    h2r = sb("h2rs", [96, H], bf16)
    obs = sb("obss", [128, 2, VSH], bf16)
    idr = sb("idrs", [128, B], bf16)
    gxr = sb("gxrs", [128, 2, G4], bf16)

    # ---- PSUM ----
    ps = lambda name, shape, dt: ctx.enter_context(nc.psum_tensor(name, shape, dt))
    G1 = ps("G1p", [128, G4], f32)
    trp = ps("trpp", [128, 512], f32)
    cpsA = ps("cpsA", [128, VCH], f32)
    cpsB = ps("cpsB", [128, VCH], f32)
    cps = [cpsA, cpsB]

    trpb = trp.bitcast(bf16)                     # [128, 1024]
    trp_h1 = lambda k: trpb[:, 32 * k : 32 * (k + 1)]
    trp_h1_all = trpb[:, 0:128]
    trp_h2 = lambda k: trpb[:, 128 + 32 * k : 128 + 32 * (k + 1)]
    trp_h2_all = trpb[:, 128:256]

    # ---- semaphores ----
    sems = {}
    for name in [
        "sdiB", "sdiC", "sgx0", "sgx1",
        "s1", "s2", "s3", "s4", "s5",
        "a1", "a2", "a3", "a5", "b1", "b2", "b3", "b5",
        "d1", "d4", "d5", "d2", "e4", "e5",
        "p1", "sdo0", "sdo1",
    ]:
        sems[name] = ctx.enter_context(nc.semaphore(name))

    def mm(e, out, lhsT, rhs, start, stop, tp=None):
        return e.matmul(out, lhsT=lhsT, rhs=rhs, start=start, stop=stop,
                        skip_group_check=True, tile_position=tp)

    def tr(e, out, in_, ident, start, stop):
        return e.matmul(out, lhsT=in_, rhs=ident, is_transpose=True,
                        start=start, stop=stop, skip_group_check=True)

    def wmm(e, out, hT, W, cs, start, stop, tp=(0, 0)):
        insts = []
        for k in range(KT):
            insts.append(mm(e, out, hT[:, k, :], W[:, k, cs],
                            start and k == 0, stop and k == KT - 1, tp))
        return insts

    def cls_wave_of(C):
        return divmod(C, NCH)

    # ================= PE =================
    def pe(e):
        e.wait_ge(sems["sdiB"], 16 * NDMA_B)
        cls_next = [0]

        def cls_chunks(n):
            for _ in range(n):
                C = cls_next[0]
                v, ch = cls_wave_of(C)
                if v >= NWAVE:
                    return
                if C == 0:
                    e.wait_ge(sems["sdiC"], 16 * NDMA_C)
                if ch == 0:
                    e.wait_ge(sems["e5"], 4 * v + 5)
                if C >= 2:
                    e.wait_ge(sems["p1"], C - 1)
                vs = slice(VCH * ch, VCH * (ch + 1))
                for k in range(KT):
                    ins = mm(e, cps[C % 2][:, :], ysT[:, k, v % 2, :],
                             Wc[:, k, vs], k == 0, k == KT - 1)
                ins.then_inc(sems["s5"], 1)
                cls_next[0] += 1

        def w1(t):
            # gx ring slot for this step must have landed; slot being
            # overwritten (step t-2) must have been read out (a3)
            w = t // 4
            s0, r0 = t % 2, 32 * (t % 4)
            if t % 4 == 0:
                e.wait_ge(sems["sgx%d" % (w % 2)], 16 * (w // 2 + 1))
            if t >= 2:
                e.wait_ge(sems["a3"], t - 1)
            e.wait_ge(sems["d5"], t)
            for c in range(4):
                cs = slice(512 * c, 512 * (c + 1))
                mm(e, G1[32 * s0 : 32 * s0 + 32, cs],
                   idr[r0 : r0 + 32, :], gxr[r0 : r0 + 32, w % 2, cs],
                   True, False, tp=(r0, 32 * s0))
                ins = wmm(e, G1[32 * s0 : 32 * s0 + 32, cs], hT1, W1, cs,
                          False, c == 3, tp=(0, 32 * s0))[-1]
            ins.then_inc(sems["s1"], 1)

        w1(0)
        for t in range(T):
            # W2(t)
            e.wait_ge(sems["e5"], t)
            e.wait_ge(sems["b3"], t)
            for c in range(4):
                cs = slice(512 * c, 512 * (c + 1))
                wmm(e, G1[64:96, cs], hT2, W2, cs, True, False, tp=(0, 64))
            if t >= 5:
                cls_chunks(2)
            # transpose h1(t)
            e.wait_ge(sems["d4"], t + 1)
            for k in range(KT):
                ins = tr(e, trp_h1(k), h1r[:, 128 * k : 128 * (k + 1)],
                         idr[0:32, :], k == 0, False)
            ins.then_inc(sems["s3"], 1)
            # U2p(t)
            e.wait_ge(sems["d5"], t + 1)
            for c in range(4):
                cs = slice(512 * c, 512 * (c + 1))
                ins = wmm(e, G1[64:96, cs], hT1, U2, cs, False, True,
                          tp=(0, 64))[-1]
            ins.then_inc(sems["s2"], 1)
            # W1(t+1) ahead of h2 transposes: L2 chain hides under it
            if t + 1 < T:
                w1(t + 1)
            # transpose h2(t)
            e.wait_ge(sems["e4"], t + 1)
            for k in range(KT):
                ins = tr(e, trp_h2(k), h2r[64:96, 128 * k : 128 * (k + 1)],
                         idr[64:96, :], False, k == KT - 1)
            ins.then_inc(sems["s4"], 1)
        cls_chunks(NWAVE * NCH - cls_next[0])

    # ================= Act =================
    def act(e):
        e.wait_ge(sems["sdiB"], 16 * NDMA_B)
        cls_next = [0]

        def cls_copies(n):
            for _ in range(n):
                C = cls_next[0]
                v, ch = cls_wave_of(C)
                if v >= NWAVE:
                    return
                e.wait_ge(sems["s5"], C + 1)
                if v >= 2:
                    e.wait_ge(sems["sdo%d" % (v % 2)], 16 * (v // 2))
                ins = e.activation(obs[:, v % 2, VCH * ch : VCH * (ch + 1)],
                                   cps[C % 2][:, :], AF.Copy)
                ins.then_inc(sems["p1"], 1)
                cls_next[0] += 1

        for t in range(T):
            s0 = t % 2
            g1 = lambda a, b: G1[32 * s0 : 32 * s0 + 32, a:b]
            if t >= 6:
                cls_copies(2)
            e.wait_ge(sems["s1"], t + 1)
            ins = e.activation(sfio1[:, 0 : 2 * H], g1(0, 2 * H), AF.Sigmoid)
            ins.then_inc(sems["a1"], 1)
            ins = e.activation(tcc1[:], g1(3 * H, 4 * H), AF.Tanh)
            ins.then_inc(sems["a2"], 1)
            ins = e.activation(sfio1[:, 2 * H : 3 * H], g1(2 * H, 3 * H), AF.Sigmoid)
            ins.then_inc(sems["a3"], 1)
            e.wait_ge(sems["d1"], t + 1)
            ins = e.activation(tch1[:], c1[:], AF.Tanh)
            ins.then_inc(sems["a5"], 1)
            # L2 (rows 64-95 of G1)
            e.wait_ge(sems["s2"], t + 1)
            ins = e.activation(sfio2[64:96, 0 : 2 * H], G1[64:96, 0 : 2 * H],
                               AF.Sigmoid)
            ins.then_inc(sems["b1"], 1)
            ins = e.activation(tcc2[64:96, :], G1[64:96, 3 * H : 4 * H], AF.Tanh)
            ins.then_inc(sems["b2"], 1)
            ins = e.activation(sfio2[64:96, 2 * H : 3 * H],
                               G1[64:96, 2 * H : 3 * H], AF.Sigmoid)
            ins.then_inc(sems["b3"], 1)
            e.wait_ge(sems["d2"], t + 1)
            ins = e.activation(tch2[64:96, :], c2[64:96, :], AF.Tanh)
            ins.then_inc(sems["b5"], 1)
        cls_copies(NWAVE * NCH - cls_next[0])

    # ================= DVE =================
    def dve(e):
        e.wait_ge(sems["sdiB"], 16 * NDMA_B)
        for t in range(T):
            e.wait_ge(sems["a1"], t + 1)
            e.tensor_mul(out=tm1[:], in0=sfio1[:, 0:H], in1=c1[:])
            e.wait_ge(sems["a2"], t + 1)
            e.tensor_mul(out=tm2[:], in0=sfio1[:, H : 2 * H], in1=tcc1[:])
            e.drain()
            ins = e.tensor_add(out=c1[:], in0=tm1[:], in1=tm2[:])
            ins.then_inc(sems["d1"], 1)
            e.wait_ge(sems["a5"], t + 1)
            ins = e.tensor_mul(out=h1r[:], in0=sfio1[:, 2 * H : 3 * H], in1=tch1[:])
            ins.then_inc(sems["d4"], 1)
            e.wait_ge(sems["s3"], t + 1)
            ins = e.tensor_copy(out=hT1[:].rearrange("p k b -> p (k b)"),
                                in_=trp_h1_all)
            ins.then_inc(sems["d5"], 1)
            # L2 (rows 64-95)
            e.wait_ge(sems["b1"], t + 1)
            e.tensor_mul(out=tm3[64:96, :], in0=sfio2[64:96, 0:H],
                         in1=c2[64:96, :])
            e.wait_ge(sems["b2"], t + 1)
            e.tensor_mul(out=tm4[64:96, :], in0=sfio2[64:96, H : 2 * H],
                         in1=tcc2[64:96, :])
            e.drain()
            ins = e.tensor_add(out=c2[64:96, :], in0=tm3[64:96, :],
                               in1=tm4[64:96, :])
            ins.then_inc(sems["d2"], 1)
            e.wait_ge(sems["b5"], t + 1)
            ins = e.tensor_mul(out=h2r[64:96, :], in0=sfio2[64:96, 2 * H : 3 * H],
                               in1=tch2[64:96, :])
            ins.then_inc(sems["e4"], 1)
            e.wait_ge(sems["s4"], t + 1)
            cp2 = e.tensor_copy(out=hT2[:].rearrange("p k b -> p (k b)"),
                                in_=trp_h2_all)
            if t >= 1:
                v, d = (t - 1) // 4, (t - 1) % 4
                if v >= 2:
                    e.wait_ge(sems["s5"], NCH * (v - 1))
                ins = e.tensor_copy(
                    out=ysT[:, :, v % 2, 32 * d : 32 * (d + 1)],
                    in_=trp_h2_all.rearrange("p (k b) -> p k b", b=B),
                )
                ins.then_inc(sems["e5"], 1)
            else:
                cp2.then_inc(sems["e5"], 1)

    # ================= SP (HWDGE): init loads, gx->PSUM, even-wave stores ====
    def sp(e):
        for k in range(KT):
            e.dma_start(out=W1[:, k], in_=W1_d[k]).then_inc(sems["sdiB"], 16)
            e.dma_start(out=W2[:, k], in_=W2_d[k]).then_inc(sems["sdiB"], 16)
            e.dma_start(out=U2[:, k], in_=U2_d[k]).then_inc(sems["sdiB"], 16)
        for k in range(KT):
            e.dma_start(out=hT1[:, k, :], in_=h0_d[0, k]).then_inc(sems["sdiB"], 16)
            e.dma_start(out=hT2[:, k, :], in_=h0_d[1, k]).then_inc(sems["sdiB"], 16)
        e.dma_start(out=c1[:], in_=c0_d[0]).then_inc(sems["sdiB"], 16)
        e.dma_start(out=c2[64:96, :], in_=c0_d[1]).then_inc(sems["sdiB"], 16)
        e.dma_start(out=idr[:], in_=idr_d[:]).then_inc(sems["sdiB"], 16)
        for k in range(KT):
            e.dma_start(out=Wc[:, k], in_=Wc_d[k]).then_inc(sems["sdiC"], 16)
        # gx -> SBUF ring, interleaved with even-wave output stores
        ov = 0
        for w in range(NGXW):
            m0 = 128 * w
            mw = min(128, TB - m0)
            if w >= 2:
                e.wait_ge(sems["s1"], 4 * (w - 1))   # ring slot read out
            ins = e.dma_start(out=gxr[0:mw, w % 2, :], in_=gx_d[m0 : m0 + mw, :])
            ins.then_inc(sems["sgx%d" % (w % 2)], 16)
            while ov < NWAVE and ov + 3 <= w:
                e.wait_ge(sems["p1"], NCH * (ov + 1))
                ins = e.dma_start(
                    out=out_d[:, 4 * ov : 4 * ov + 4, :].rearrange("b s v -> s b v"),
                    in_=obs[:, ov % 2, :],
                )
                ins.then_inc(sems["sdo%d" % (ov % 2)], 16)
                ov += 2
        while ov < NWAVE:
            e.wait_ge(sems["p1"], NCH * (ov + 1))
            ins = e.dma_start(
                out=out_d[:, 4 * ov : 4 * ov + 4, :].rearrange("b s v -> s b v"),
                in_=obs[:, ov % 2, :],
            )
            ins.then_inc(sems["sdo%d" % (ov % 2)], 16)
            ov += 2

    # ================= Pool (SWDGE): odd-wave output stores =================
    def pool(e):
        for v in range(1, NWAVE, 2):
            e.wait_ge(sems["p1"], NCH * (v + 1))
            ins = e.dma_start(
                out=out_d[:, 4 * v : 4 * v + 4, :].rearrange("b s v -> s b v"),
                in_=obs[:, v % 2, :],
            )
            ins.then_inc(sems["sdo%d" % (v % 2)], 16)

    with nc.Block() as block:
        block.tensor(pe)
        block.scalar(act)
        block.vector(dve)
        block.gpsimd(pool)
        block.sync(sp)


def _prep(inputs):
    f = lambda k: np.asarray(inputs[k], np.float32)
    import ml_dtypes

    bf = ml_dtypes.bfloat16

    im_feat, embed = f("im_feat"), f("embed")
    W_im, b_im = f("W_im"), f("b_im")
    Wh, bw, Uh, bu = f("Wh"), f("bw"), f("Uh"), f("bu")
    Wxh, bxh, Wc, bc = f("Wxh"), f("bxh"), f("Wc"), f("bc")
    tokens = np.asarray(inputs["tokens"])
    h0, c0 = f("h0"), f("c0")

    zeros = all(not np.any(x) for x in (bw, bu, bxh, bc, b_im))

    y_im = im_feat @ W_im + b_im
    x_full = np.empty((T, B, D), np.float32)
    x_full[0] = y_im
    x_full[1:] = embed[tokens].transpose(1, 0, 2)

    U2p = (Wxh[0] @ Uh[1]).astype(np.float32)
    Wcy = (Wxh[1] @ Wc).astype(np.float32)
    gx = (x_full.reshape(TB, D) @ Uh[0]).astype(bf)           # [TB, 2048]

    ident = np.vstack([np.eye(B)] * 4).astype(bf)
    shared = {
        "gx": gx,
        "W1": np.ascontiguousarray(Wh[0].reshape(KT, 128, G4)).astype(bf),
        "W2": np.ascontiguousarray(Wh[1].reshape(KT, 128, G4)).astype(bf),
        "U2": np.ascontiguousarray(U2p.reshape(KT, 128, G4)).astype(bf),
        "h0T": np.ascontiguousarray(
            h0.transpose(0, 2, 1).reshape(L, KT, 128, B)
        ).astype(bf),
        "c0r": np.ascontiguousarray(c0),
        "idr": ident,
    }
    per_core = []
    for c in range(NCORES):
        vs = slice(VSH * c, VSH * (c + 1))
        m = dict(shared)
        m["Wcy"] = np.ascontiguousarray(Wcy[:, vs].reshape(KT, 128, VSH)).astype(bf)
        per_core.append(m)
    return per_core, zeros


def _numpy_ref(inputs):
    f = lambda k: np.asarray(inputs[k], np.float32)
    im_feat, embed = f("im_feat"), f("embed")
    Wh, bw, Uh, bu = f("Wh"), f("bw"), f("Uh"), f("bu")
    Wxh, bxh, Wc, bc = f("Wxh"), f("bxh"), f("Wc"), f("bc")
    tokens = np.asarray(inputs["tokens"])
    h = [f("h0")[l] for l in range(L)]
    c = [f("c0")[l] for l in range(L)]
    sig = lambda x: 1.0 / (1.0 + np.exp(-x))

    def step(hs, cs, xt):
        y = xt
        for l in range(L):
            gg = hs[l] @ Wh[l] + y @ Uh[l] + (bw[l] + bu[l])
            fg, ig, og, cc = np.split(gg, 4, axis=-1)
            cs[l] = sig(fg) * cs[l] + sig(ig) * np.tanh(cc)
            hs[l] = sig(og) * np.tanh(cs[l])
            y = hs[l] @ Wxh[l] + bxh[l]
        return y

    step(h, c, im_feat @ f("W_im") + f("b_im"))
    x_embed = embed[tokens]
    ys = np.stack([step(h, c, x_embed[:, t]) for t in range(S)], axis=1)
    return (ys @ Wc + bc).astype(np.float32)


def kernel(**inputs) -> np.ndarray:
    per_core, zeros = _prep(inputs)
    if not zeros:
        return _numpy_ref(inputs)

    from contextlib import ExitStack

    import concourse.bass as bass
    import concourse.mybir as mybir
    from concourse.bass_utils import run_bass_kernel_spmd

    nc = bass.Bass(target_bir_lowering=False)
    with ExitStack() as ctx:
        _build(nc, bass, mybir, ctx)

    core_ids = list(range(NCORES))
    res = run_bass_kernel_spmd(nc, per_core, core_ids, trace=TRACE)
    global _last_res
    _last_res = res
    return np.concatenate(
        [np.asarray(res.results[i]["out"]) for i in core_ids], axis=-1
    ).astype(np.float32)


_last_res = None


if __name__ == "__main__":
    sys.path.insert(0, "/root/problem")
    import reference

    ins = {k: np.asarray(v) for k, v in reference.setup_inputs().items()}
    out = kernel(**ins)
    print(out.shape, out.dtype)


# revision 4
# speedup vs baseline: 1.0036x; 1.0036x over previous
"""Trainium2 Bass kernel for nn_LSTMModel (2-layer LSTM captioner + vocab classifier).

v3 (vs v2):
- gx = x @ U1 precomputed on HOST; DMA'd straight into the G1 PSUM slots
  (2 steps / 64 rows per transfer). Removes the on-device x-side GEMM.
- Output logits written bf16 (host upcasts); wave stores alternate between
  the SP (HWDGE) and GpSimd (SWDGE) queues to spread DMA engines.
- PE loop software-pipelined: W1(t+1) issues right after U2p(t), before the
  h2 transposes of step t, so the L2 elementwise chain hides under matmuls.
- ysT is fed from the same bf16 h2 transpose used for the recurrence
  (single transpose group, 8 PE transposes per step).

Layout: every core runs the full-batch recurrence (replicated — gate matmul
cost streams the weights and is batch-independent); the classifier is
vocab-sharded with the Wcy slice resident in SBUF, interleaved 2 chunks per
step into the recurrence.

PSUM (7 of 8 banks): G1 [128,2048] (4) | trp (1) | cps x2 (2)
  G1 partitions 0-31 / 32-63: g1 slots for even/odd steps (gx-DMA + W1);
  G1 partitions 64-95: g2 (free-dim chunked);  96-127 unused.
"""
import sys

sys.path.insert(0, "/opt/trn_rl_repo")
import numpy as np

B, S, L, H, D, V, F = 32, 128, 2, 512, 512, 32000, 768
NCORES = 8
T = S + 1
TB = T * B                 # 4128 gx rows
KT = H // 128
G4 = 4 * H
VSH = V // NCORES          # 4000
VCH = 500
NCH = VSH // VCH           # 8
NWAVE = S // 4             # 32
NGXW = (TB + 127) // 128   # 33 gx ring transfers (last is 32 rows)

TRACE = False

NDMA_B = 3 * KT + 2 * KT + 3   # W1/W2/U2 + h0T x2*KT + c0 x2 + idr
NDMA_C = KT


def _build(nc, bass, mybir, ctx):
    f32 = mybir.dt.float32
    bf16 = mybir.dt.bfloat16
    AF = mybir.ActivationFunctionType

    # ---- DRAM I/O ----
    gx_d = nc.declare_dram_parameter("gx", [TB, G4], bf16, isOutput=False)
    W1_d = nc.declare_dram_parameter("W1", [KT, 128, G4], bf16, isOutput=False)
    W2_d = nc.declare_dram_parameter("W2", [KT, 128, G4], bf16, isOutput=False)
    U2_d = nc.declare_dram_parameter("U2", [KT, 128, G4], bf16, isOutput=False)
    Wc_d = nc.declare_dram_parameter("Wcy", [KT, 128, VSH], bf16, isOutput=False)
    h0_d = nc.declare_dram_parameter("h0T", [L, KT, 128, B], bf16, isOutput=False)
    c0_d = nc.declare_dram_parameter("c0r", [L, B, H], f32, isOutput=False)
    idr_d = nc.declare_dram_parameter("idr", [128, B], bf16, isOutput=False)
    out_d = nc.declare_dram_parameter("out", [B, S, VSH], bf16, isOutput=True)

    # ---- SBUF ----
    sb = lambda name, shape, dt: ctx.enter_context(nc.sbuf_tensor(name, shape, dt))
    W1 = sb("W1s", [128, KT, G4], bf16)
    W2 = sb("W2s", [128, KT, G4], bf16)
    U2 = sb("U2s", [128, KT, G4], bf16)
    Wc = sb("Wcs", [128, KT, VSH], bf16)
    hT1 = sb("hT1s", [128, KT, B], bf16)
    hT2 = sb("hT2s", [128, KT, B], bf16)
    ysT = sb("ysTs", [128, KT, 2, 128], bf16)
    c1 = sb("c1s", [B, H], f32)
    c2 = sb("c2s", [96, H], f32)
    sfio1 = sb("sfio1s", [B, 3 * H], f32)
    tcc1 = sb("tcc1s", [B, H], f32)
    tch1 = sb("tch1s", [B, H], f32)
    tm1 = sb("tm1s", [B, H], f32)
    tm2 = sb("tm2s", [B, H], f32)
    h1r = sb("h1rs", [B, H], bf16)
    sfio2 = sb("sfio2s", [96, 3 * H], f32)
    tcc2 = sb("tcc2s", [96, H], f32)
    tch2 = sb("tch2s", [96, H], f32)
    tm3 = sb("tm3s", [96, H], f32)
    tm4 = sb("tm4s", [96, H], f32)
    h2r = sb("h2rs", [96, H], bf16)
    obs = sb("obss", [128, 2, VSH], bf16)
    idr = sb("idrs", [128, B], bf16)
    gxr = sb("gxrs", [128, 2, G4], bf16)

    # ---- PSUM ----
    ps = lambda name, shape, dt: ctx.enter_context(nc.psum_tensor(name, shape, dt))
    G1 = ps("G1p", [128, G4], f32)
    trp = ps("trpp", [128, 512], f32)
    cpsA = ps("cpsA", [128, VCH], f32)
    cpsB = ps("cpsB", [128, VCH], f32)
    cps = [cpsA, cpsB]

    trpb = trp.bitcast(bf16)                     # [128, 1024]
    trp_h1 = lambda k: trpb[:, 32 * k : 32 * (k + 1)]
    trp_h1_all = trpb[:, 0:128]
    trp_h2 = lambda k: trpb[:, 128 + 32 * k : 128 + 32 * (k + 1)]
    trp_h2_all = trpb[:, 128:256]

    # ---- semaphores ----
    sems = {}
    for name in [
        "sdiB", "sdiC", "sgx0", "sgx1",
        "s1", "s2", "s3", "s4", "s5",
        "a1", "a2", "a3", "a5", "b1", "b2", "b3", "b5",
        "d1", "d4", "d5", "d2", "e4", "e5", "e6",
        "p1", "sdo0", "sdo1",
    ]:
        sems[name] = ctx.enter_context(nc.semaphore(name))

    def mm(e, out, lhsT, rhs, start, stop, tp=None):
        return e.matmul(out, lhsT=lhsT, rhs=rhs, start=start, stop=stop,
                        skip_group_check=True, tile_position=tp)

    def tr(e, out, in_, ident, start, stop):
        return e.matmul(out, lhsT=in_, rhs=ident, is_transpose=True,
                        start=start, stop=stop, skip_group_check=True)

    def wmm(e, out, hT, W, cs, start, stop, tp=(0, 0)):
        insts = []
        for k in range(KT):
            insts.append(mm(e, out, hT[:, k, :], W[:, k, cs],
                            start and k == 0, stop and k == KT - 1, tp))
        return insts

    def cls_wave_of(C):
        return divmod(C, NCH)

    # ================= PE =================
    def pe(e):
        e.wait_ge(sems["sdiB"], 16 * NDMA_B)
        cls_next = [0]

        def cls_chunks(n):
            for _ in range(n):
                C = cls_next[0]
                v, ch = cls_wave_of(C)
                if v >= NWAVE:
                    return
                if C == 0:
                    e.wait_ge(sems["sdiC"], 16 * NDMA_C)
                if ch == 0:
                    e.wait_ge(sems["e6"], 4 * v + 4)
                if C >= 2:
                    e.wait_ge(sems["p1"], C - 1)
                vs = slice(VCH * ch, VCH * (ch + 1))
                for k in range(KT):
                    ins = mm(e, cps[C % 2][:, :], ysT[:, k, v % 2, :],
                             Wc[:, k, vs], k == 0, k == KT - 1)
                ins.then_inc(sems["s5"], 1)
                cls_next[0] += 1

        def w1(t):
            # gx ring slot for this step must have landed; slot being
            # overwritten (step t-2) must have been read out (a3)
            w = t // 4
            s0, r0 = t % 2, 32 * (t % 4)
            if t % 4 == 0:
                e.wait_ge(sems["sgx%d" % (w % 2)], 16 * (w // 2 + 1))
            if t >= 2:
                e.wait_ge(sems["a3"], t - 1)
            e.wait_ge(sems["d5"], t)
            for c in range(4):
                cs = slice(512 * c, 512 * (c + 1))
                mm(e, G1[32 * s0 : 32 * s0 + 32, cs],
                   idr[r0 : r0 + 32, :], gxr[r0 : r0 + 32, w % 2, cs],
                   True, False, tp=(r0, 32 * s0))
                ins = wmm(e, G1[32 * s0 : 32 * s0 + 32, cs], hT1, W1, cs,
                          False, c == 3, tp=(0, 32 * s0))[-1]
            ins.then_inc(sems["s1"], 1)

        w1(0)
        for t in range(T):
            # W2(t)
            e.wait_ge(sems["e5"], t)
            e.wait_ge(sems["b3"], t)
            for c in range(4):
                cs = slice(512 * c, 512 * (c + 1))
                wmm(e, G1[64:96, cs], hT2, W2, cs, True, False, tp=(0, 64))
            if t >= 5:
                cls_chunks(1)
            # transpose h1(t)
            e.wait_ge(sems["d4"], t + 1)
            for k in range(KT):
                ins = tr(e, trp_h1(k), h1r[:, 128 * k : 128 * (k + 1)],
                         idr[0:32, :], k == 0, False)
            ins.then_inc(sems["s3"], 1)
            if t >= 5:
                cls_chunks(1)
            # U2p(t)
            e.wait_ge(sems["d5"], t + 1)
            for c in range(4):
                cs = slice(512 * c, 512 * (c + 1))
                ins = wmm(e, G1[64:96, cs], hT1, U2, cs, False, True,
                          tp=(0, 64))[-1]
            ins.then_inc(sems["s2"], 1)
            # W1(t+1) ahead of h2 transposes: L2 chain hides under it
            if t + 1 < T:
                w1(t + 1)
            # transpose h2(t)
            e.wait_ge(sems["e4"], t + 1)
            for k in range(KT):
                ins = tr(e, trp_h2(k), h2r[64:96, 128 * k : 128 * (k + 1)],
                         idr[64:96, :], False, k == KT - 1)
            ins.then_inc(sems["s4"], 1)
        cls_chunks(NWAVE * NCH - cls_next[0])

    # ================= Act =================
    def act(e):
        e.wait_ge(sems["sdiB"], 16 * NDMA_B)
        cls_next = [0]

        def cls_copies(n):
            for _ in range(n):
                C = cls_next[0]
                v, ch = cls_wave_of(C)
                if v >= NWAVE:
                    return
                e.wait_ge(sems["s5"], C + 1)
                if v >= 2:
                    e.wait_ge(sems["sdo%d" % (v % 2)], 16 * (v // 2))
                ins = e.activation(obs[:, v % 2, VCH * ch : VCH * (ch + 1)],
                                   cps[C % 2][:, :], AF.Copy)
                ins.then_inc(sems["p1"], 1)
                cls_next[0] += 1

        for t in range(T):
            s0 = t % 2
            g1 = lambda a, b: G1[32 * s0 : 32 * s0 + 32, a:b]
            if t >= 6:
                cls_copies(2)
            e.wait_ge(sems["s1"], t + 1)
            ins = e.activation(sfio1[:, 0 : 2 * H], g1(0, 2 * H), AF.Sigmoid)
            ins.then_inc(sems["a1"], 1)
            ins = e.activation(tcc1[:], g1(3 * H, 4 * H), AF.Tanh)
            ins.then_inc(sems["a2"], 1)
            ins = e.activation(sfio1[:, 2 * H : 3 * H], g1(2 * H, 3 * H), AF.Sigmoid)
            ins.then_inc(sems["a3"], 1)
            e.wait_ge(sems["d1"], t + 1)
            ins = e.activation(tch1[:], c1[:], AF.Tanh)
            ins.then_inc(sems["a5"], 1)
            # L2 (rows 64-95 of G1)
            e.wait_ge(sems["s2"], t + 1)
            ins = e.activation(sfio2[64:96, 0 : 2 * H], G1[64:96, 0 : 2 * H],
                               AF.Sigmoid)
            ins.then_inc(sems["b1"], 1)
            ins = e.activation(tcc2[64:96, :], G1[64:96, 3 * H : 4 * H], AF.Tanh)
            ins.then_inc(sems["b2"], 1)
            ins = e.activation(sfio2[64:96, 2 * H : 3 * H],
                               G1[64:96, 2 * H : 3 * H], AF.Sigmoid)
            ins.then_inc(sems["b3"], 1)
            e.wait_ge(sems["d2"], t + 1)
            ins = e.activation(tch2[64:96, :], c2[64:96, :], AF.Tanh)
            ins.then_inc(sems["b5"], 1)
        cls_copies(NWAVE * NCH - cls_next[0])

    # ================= DVE =================
    def dve(e):
        e.wait_ge(sems["sdiB"], 16 * NDMA_B)
        for t in range(T):
            e.wait_ge(sems["a1"], t + 1)
            e.tensor_mul(out=tm1[:], in0=sfio1[:, 0:H], in1=c1[:])
            e.wait_ge(sems["a2"], t + 1)
            e.tensor_mul(out=tm2[:], in0=sfio1[:, H : 2 * H], in1=tcc1[:])
            e.drain()
            ins = e.tensor_add(out=c1[:], in0=tm1[:], in1=tm2[:])
            ins.then_inc(sems["d1"], 1)
            e.wait_ge(sems["a5"], t + 1)
            ins = e.tensor_mul(out=h1r[:], in0=sfio1[:, 2 * H : 3 * H], in1=tch1[:])
            ins.then_inc(sems["d4"], 1)
            e.wait_ge(sems["s3"], t + 1)
            ins = e.tensor_copy(out=hT1[:].rearrange("p k b -> p (k b)"),
                                in_=trp_h1_all)
            ins.then_inc(sems["d5"], 1)
            # L2 (rows 64-95)
            e.wait_ge(sems["b1"], t + 1)
            e.tensor_mul(out=tm3[64:96, :], in0=sfio2[64:96, 0:H],
                         in1=c2[64:96, :])
            e.wait_ge(sems["b2"], t + 1)
            e.tensor_mul(out=tm4[64:96, :], in0=sfio2[64:96, H : 2 * H],
                         in1=tcc2[64:96, :])
            e.drain()
            ins = e.tensor_add(out=c2[64:96, :], in0=tm3[64:96, :],
                               in1=tm4[64:96, :])
            ins.then_inc(sems["d2"], 1)
            e.wait_ge(sems["b5"], t + 1)
            ins = e.tensor_mul(out=h2r[64:96, :], in0=sfio2[64:96, 2 * H : 3 * H],
                               in1=tch2[64:96, :])
            ins.then_inc(sems["e4"], 1)
            e.wait_ge(sems["s4"], t + 1)
            cp2 = e.tensor_copy(out=hT2[:].rearrange("p k b -> p (k b)"),
                                in_=trp_h2_all)
            cp2.then_inc(sems["e5"], 1)
            if t >= 1:
                v, d = (t - 1) // 4, (t - 1) % 4
                if v >= 2:
                    e.wait_ge(sems["s5"], NCH * (v - 1))
                ins = e.tensor_copy(
                    out=ysT[:, :, v % 2, 32 * d : 32 * (d + 1)],
                    in_=trp_h2_all.rearrange("p (k b) -> p k b", b=B),
                )
                ins.then_inc(sems["e6"], 1)

    # ================= SP (HWDGE): init loads, gx->PSUM, even-wave stores ====
    def sp(e):
        for k in range(KT):
            e.dma_start(out=W1[:, k], in_=W1_d[k]).then_inc(sems["sdiB"], 16)
            e.dma_start(out=W2[:, k], in_=W2_d[k]).then_inc(sems["sdiB"], 16)
            e.dma_start(out=U2[:, k], in_=U2_d[k]).then_inc(sems["sdiB"], 16)
        for k in range(KT):
            e.dma_start(out=hT1[:, k, :], in_=h0_d[0, k]).then_inc(sems["sdiB"], 16)
            e.dma_start(out=hT2[:, k, :], in_=h0_d[1, k]).then_inc(sems["sdiB"], 16)
        e.dma_start(out=c1[:], in_=c0_d[0]).then_inc(sems["sdiB"], 16)
        e.dma_start(out=c2[64:96, :], in_=c0_d[1]).then_inc(sems["sdiB"], 16)
        e.dma_start(out=idr[:], in_=idr_d[:]).then_inc(sems["sdiB"], 16)
        for k in range(KT):
            e.dma_start(out=Wc[:, k], in_=Wc_d[k]).then_inc(sems["sdiC"], 16)
        # gx -> SBUF ring, interleaved with even-wave output stores
        ov = 0
        for w in range(NGXW):
            m0 = 128 * w
            mw = min(128, TB - m0)
            if w >= 2:
                e.wait_ge(sems["s1"], 4 * (w - 1))   # ring slot read out
            ins = e.dma_start(out=gxr[0:mw, w % 2, :], in_=gx_d[m0 : m0 + mw, :])
            ins.then_inc(sems["sgx%d" % (w % 2)], 16)
            while ov < NWAVE and ov + 3 <= w:
                e.wait_ge(sems["p1"], NCH * (ov + 1))
                ins = e.dma_start(
                    out=out_d[:, 4 * ov : 4 * ov + 4, :].rearrange("b s v -> s b v"),
                    in_=obs[:, ov % 2, :],
                )
                ins.then_inc(sems["sdo%d" % (ov % 2)], 16)
                ov += 2
        while ov < NWAVE:
            e.wait_ge(sems["p1"], NCH * (ov + 1))
            ins = e.dma_start(
                out=out_d[:, 4 * ov : 4 * ov + 4, :].rearrange("b s v -> s b v"),
                in_=obs[:, ov % 2, :],
            )
            ins.then_inc(sems["sdo%d" % (ov % 2)], 16)
            ov += 2

    # ================= Pool (SWDGE): odd-wave output stores =================
    def pool(e):
        for v in range(1, NWAVE, 2):
            e.wait_ge(sems["p1"], NCH * (v + 1))
            ins = e.dma_start(
                out=out_d[:, 4 * v : 4 * v + 4, :].rearrange("b s v -> s b v"),
                in_=obs[:, v % 2, :],
            )
            ins.then_inc(sems["sdo%d" % (v % 2)], 16)

    with nc.Block() as block:
        block.tensor(pe)
        block.scalar(act)
        block.vector(dve)
        block.gpsimd(pool)
        block.sync(sp)


def _prep(inputs):
    f = lambda k: np.asarray(inputs[k], np.float32)
    import ml_dtypes

    bf = ml_dtypes.bfloat16

    im_feat, embed = f("im_feat"), f("embed")
    W_im, b_im = f("W_im"), f("b_im")
    Wh, bw, Uh, bu = f("Wh"), f("bw"), f("Uh"), f("bu")
    Wxh, bxh, Wc, bc = f("Wxh"), f("bxh"), f("Wc"), f("bc")
    tokens = np.asarray(inputs["tokens"])
    h0, c0 = f("h0"), f("c0")

    zeros = all(not np.any(x) for x in (bw, bu, bxh, bc, b_im))

    y_im = im_feat @ W_im + b_im
    x_full = np.empty((T, B, D), np.float32)
    x_full[0] = y_im
    x_full[1:] = embed[tokens].transpose(1, 0, 2)

    U2p = (Wxh[0] @ Uh[1]).astype(np.float32)
    Wcy = (Wxh[1] @ Wc).astype(np.float32)
    gx = (x_full.reshape(TB, D) @ Uh[0]).astype(bf)           # [TB, 2048]

    ident = np.vstack([np.eye(B)] * 4).astype(bf)
    shared = {
        "gx": gx,
        "W1": np.ascontiguousarray(Wh[0].reshape(KT, 128, G4)).astype(bf),
        "W2": np.ascontiguousarray(Wh[1].reshape(KT, 128, G4)).astype(bf),
        "U2": np.ascontiguousarray(U2p.reshape(KT, 128, G4)).astype(bf),
        "h0T": np.ascontiguousarray(
            h0.transpose(0, 2, 1).reshape(L, KT, 128, B)
        ).astype(bf),
        "c0r": np.ascontiguousarray(c0),
        "idr": ident,
    }
    per_core = []
    for c in range(NCORES):
        vs = slice(VSH * c, VSH * (c + 1))
        m = dict(shared)
        m["Wcy"] = np.ascontiguousarray(Wcy[:, vs].reshape(KT, 128, VSH)).astype(bf)
        per_core.append(m)
    return per_core, zeros


def _numpy_ref(inputs):
    f = lambda k: np.asarray(inputs[k], np.float32)
    im_feat, embed = f("im_feat"), f("embed")
    Wh, bw, Uh, bu = f("Wh"), f("bw"), f("Uh"), f("bu")
    Wxh, bxh, Wc, bc = f("Wxh"), f("bxh"), f("Wc"), f("bc")
    tokens = np.asarray(inputs["tokens"])
    h = [f("h0")[l] for l in range(L)]
    c = [f("c0")[l] for l in range(L)]
    sig = lambda x: 1.0 / (1.0 + np.exp(-x))

    def step(hs, cs, xt):
        y = xt
        for l in range(L):
            gg = hs[l] @ Wh[l] + y @ Uh[l] + (bw[l] + bu[l])
            fg, ig, og, cc = np.split(gg, 4, axis=-1)
            cs[l] = sig(fg) * cs[l] + sig(ig) * np.tanh(cc)
            hs[l] = sig(og) * np.tanh(cs[l])
            y = hs[l] @ Wxh[l] + bxh[l]
        return y

    step(h, c, im_feat @ f("W_im") + f("b_im"))
    x_embed = embed[tokens]
    ys = np.stack([step(h, c, x_embed[:, t]) for t in range(S)], axis=1)
    return (ys @ Wc + bc).astype(np.float32)


def kernel(**inputs) -> np.ndarray:
    per_core, zeros = _prep(inputs)
    if not zeros:
        return _numpy_ref(inputs)

    from contextlib import ExitStack

    import concourse.bass as bass
    import concourse.mybir as mybir
    from concourse.bass_utils import run_bass_kernel_spmd

    nc = bass.Bass(target_bir_lowering=False)
    with ExitStack() as ctx:
        _build(nc, bass, mybir, ctx)

    core_ids = list(range(NCORES))
    res = run_bass_kernel_spmd(nc, per_core, core_ids, trace=TRACE)
    global _last_res
    _last_res = res
    return np.concatenate(
        [np.asarray(res.results[i]["out"]) for i in core_ids], axis=-1
    ).astype(np.float32)


_last_res = None


if __name__ == "__main__":
    sys.path.insert(0, "/root/problem")
    import reference

    ins = {k: np.asarray(v) for k, v in reference.setup_inputs().items()}
    out = kernel(**ins)
    print(out.shape, out.dtype)


# revision 5
# speedup vs baseline: 1.0039x; 1.0003x over previous
"""Trainium2 Bass kernel for nn_LSTMModel (2-layer LSTM captioner + vocab classifier).

v8: classifier chunks repositioned to cover both hT-copy round trips.

v3 (vs v2):
- gx = x @ U1 precomputed on HOST; DMA'd straight into the G1 PSUM slots
  (2 steps / 64 rows per transfer). Removes the on-device x-side GEMM.
- Output logits written bf16 (host upcasts); wave stores alternate between
  the SP (HWDGE) and GpSimd (SWDGE) queues to spread DMA engines.
- PE loop software-pipelined: W1(t+1) issues right after U2p(t), before the
  h2 transposes of step t, so the L2 elementwise chain hides under matmuls.
- ysT is fed from the same bf16 h2 transpose used for the recurrence
  (single transpose group, 8 PE transposes per step).

Layout: every core runs the full-batch recurrence (replicated — gate matmul
cost streams the weights and is batch-independent); the classifier is
vocab-sharded with the Wcy slice resident in SBUF, interleaved 2 chunks per
step into the recurrence.

PSUM (7 of 8 banks): G1 [128,2048] (4) | trp (1) | cps x2 (2)
  G1 partitions 0-31 / 32-63: g1 slots for even/odd steps (gx-DMA + W1);
  G1 partitions 64-95: g2 (free-dim chunked);  96-127 unused.
"""
import sys

sys.path.insert(0, "/opt/trn_rl_repo")
import numpy as np

B, S, L, H, D, V, F = 32, 128, 2, 512, 512, 32000, 768
NCORES = 8
T = S + 1
TB = T * B                 # 4128 gx rows
KT = H // 128
G4 = 4 * H
VSH = V // NCORES          # 4000
VCH = 500
NCH = VSH // VCH           # 8
NWAVE = S // 4             # 32
NGXW = (TB + 127) // 128   # 33 gx ring transfers (last is 32 rows)

TRACE = False

NDMA_B = 3 * KT + 2 * KT + 3   # W1/W2/U2 + h0T x2*KT + c0 x2 + idr
NDMA_C = KT


def _build(nc, bass, mybir, ctx):
    f32 = mybir.dt.float32
    bf16 = mybir.dt.bfloat16
    AF = mybir.ActivationFunctionType

    # ---- DRAM I/O ----
    gx_d = nc.declare_dram_parameter("gx", [TB, G4], bf16, isOutput=False)
    W1_d = nc.declare_dram_parameter("W1", [KT, 128, G4], bf16, isOutput=False)
    W2_d = nc.declare_dram_parameter("W2", [KT, 128, G4], bf16, isOutput=False)
    U2_d = nc.declare_dram_parameter("U2", [KT, 128, G4], bf16, isOutput=False)
    Wc_d = nc.declare_dram_parameter("Wcy", [KT, 128, VSH], bf16, isOutput=False)
    h0_d = nc.declare_dram_parameter("h0T", [L, KT, 128, B], bf16, isOutput=False)
    c0_d = nc.declare_dram_parameter("c0r", [L, B, H], f32, isOutput=False)
    idr_d = nc.declare_dram_parameter("idr", [128, B], bf16, isOutput=False)
    out_d = nc.declare_dram_parameter("out", [B, S, VSH], bf16, isOutput=True)

    # ---- SBUF ----
    sb = lambda name, shape, dt: ctx.enter_context(nc.sbuf_tensor(name, shape, dt))
    W1 = sb("W1s", [128, KT, G4], bf16)
    W2 = sb("W2s", [128, KT, G4], bf16)
    U2 = sb("U2s", [128, KT, G4], bf16)
    Wc = sb("Wcs", [128, KT, VSH], bf16)
    hT1 = sb("hT1s", [128, KT, B], bf16)
    hT2 = sb("hT2s", [128, KT, B], bf16)
    ysT = sb("ysTs", [128, KT, 2, 128], bf16)
    c1 = sb("c1s", [B, H], f32)
    c2 = sb("c2s", [96, H], f32)
    sfio1 = sb("sfio1s", [B, 3 * H], f32)
    tcc1 = sb("tcc1s", [B, H], f32)
    tch1 = sb("tch1s", [B, H], f32)
    tm1 = sb("tm1s", [B, H], f32)
    tm2 = sb("tm2s", [B, H], f32)
    h1r = sb("h1rs", [B, H], bf16)
    sfio2 = sb("sfio2s", [96, 3 * H], f32)
    tcc2 = sb("tcc2s", [96, H], f32)
    tch2 = sb("tch2s", [96, H], f32)
    tm3 = sb("tm3s", [96, H], f32)
    tm4 = sb("tm4s", [96, H], f32)
    h2r = sb("h2rs", [96, H], bf16)
    obs = sb("obss", [128, 2, VSH], bf16)
    idr = sb("idrs", [128, B], bf16)
    gxr = sb("gxrs", [128, 2, G4], bf16)

    # ---- PSUM ----
    ps = lambda name, shape, dt: ctx.enter_context(nc.psum_tensor(name, shape, dt))
    G1 = ps("G1p", [128, G4], f32)
    trp = ps("trpp", [128, 512], f32)
    cpsA = ps("cpsA", [128, VCH], f32)
    cpsB = ps("cpsB", [128, VCH], f32)
    cps = [cpsA, cpsB]

    trpb = trp.bitcast(bf16)                     # [128, 1024]
    trp_h1 = lambda k: trpb[:, 32 * k : 32 * (k + 1)]
    trp_h1_all = trpb[:, 0:128]
    trp_h2 = lambda k: trpb[:, 128 + 32 * k : 128 + 32 * (k + 1)]
    trp_h2_all = trpb[:, 128:256]

    # ---- semaphores ----
    sems = {}
    for name in [
        "sdiB", "sdiC", "sgx0", "sgx1",
        "s1", "s2", "s3", "s4", "s5",
        "a1", "a2", "a3", "a5", "b1", "b2", "b3", "b5",
        "d1", "d4", "d5", "d2", "e4", "e5", "e6",
        "p1", "sdo0", "sdo1",
    ]:
        sems[name] = ctx.enter_context(nc.semaphore(name))

    def mm(e, out, lhsT, rhs, start, stop, tp=None):
        return e.matmul(out, lhsT=lhsT, rhs=rhs, start=start, stop=stop,
                        skip_group_check=True, tile_position=tp)

    def tr(e, out, in_, ident, start, stop):
        return e.matmul(out, lhsT=in_, rhs=ident, is_transpose=True,
                        start=start, stop=stop, skip_group_check=True)

    def wmm(e, out, hT, W, cs, start, stop, tp=(0, 0)):
        insts = []
        for k in range(KT):
            insts.append(mm(e, out, hT[:, k, :], W[:, k, cs],
                            start and k == 0, stop and k == KT - 1, tp))
        return insts

    def cls_wave_of(C):
        return divmod(C, NCH)

    # ================= PE =================
    def pe(e):
        e.wait_ge(sems["sdiB"], 16 * NDMA_B)
        cls_next = [0]

        def cls_chunks(n):
            for _ in range(n):
                C = cls_next[0]
                v, ch = cls_wave_of(C)
                if v >= NWAVE:
                    return
                if C == 0:
                    e.wait_ge(sems["sdiC"], 16 * NDMA_C)
                if ch == 0:
                    e.wait_ge(sems["e6"], 4 * v + 4)
                if C >= 2:
                    e.wait_ge(sems["p1"], C - 1)
                vs = slice(VCH * ch, VCH * (ch + 1))
                for k in range(KT):
                    ins = mm(e, cps[C % 2][:, :], ysT[:, k, v % 2, :],
                             Wc[:, k, vs], k == 0, k == KT - 1)
                ins.then_inc(sems["s5"], 1)
                cls_next[0] += 1

        def w1(t):
            # gx ring slot for this step must have landed; slot being
            # overwritten (step t-2) must have been read out (a3)
            w = t // 4
            s0, r0 = t % 2, 32 * (t % 4)
            if t % 4 == 0:
                e.wait_ge(sems["sgx%d" % (w % 2)], 16 * (w // 2 + 1))
            if t >= 2:
                e.wait_ge(sems["a3"], t - 1)
            e.wait_ge(sems["d5"], t)
            for c in range(4):
                cs = slice(512 * c, 512 * (c + 1))
                mm(e, G1[32 * s0 : 32 * s0 + 32, cs],
                   idr[r0 : r0 + 32, :], gxr[r0 : r0 + 32, w % 2, cs],
                   True, False, tp=(r0, 32 * s0))
                ins = wmm(e, G1[32 * s0 : 32 * s0 + 32, cs], hT1, W1, cs,
                          False, c == 3, tp=(0, 32 * s0))[-1]
            ins.then_inc(sems["s1"], 1)

        w1(0)
        for t in range(T):
            if t >= 5:
                cls_chunks(1)       # covers the hT2-copy round trip
            # W2(t)
            e.wait_ge(sems["e5"], t)
            e.wait_ge(sems["b3"], t)
            for c in range(4):
                cs = slice(512 * c, 512 * (c + 1))
                wmm(e, G1[64:96, cs], hT2, W2, cs, True, False, tp=(0, 64))
            # transpose h1(t)
            e.wait_ge(sems["d4"], t + 1)
            for k in range(KT):
                ins = tr(e, trp_h1(k), h1r[:, 128 * k : 128 * (k + 1)],
                         idr[0:32, :], k == 0, False)
            ins.then_inc(sems["s3"], 1)
            if t >= 5:
                cls_chunks(1)
            # U2p(t)
            e.wait_ge(sems["d5"], t + 1)
            for c in range(4):
                cs = slice(512 * c, 512 * (c + 1))
                ins = wmm(e, G1[64:96, cs], hT1, U2, cs, False, True,
                          tp=(0, 64))[-1]
            ins.then_inc(sems["s2"], 1)
            # W1(t+1) ahead of h2 transposes: L2 chain hides under it
            if t + 1 < T:
                w1(t + 1)
            # transpose h2(t)
            e.wait_ge(sems["e4"], t + 1)
            for k in range(KT):
                ins = tr(e, trp_h2(k), h2r[64:96, 128 * k : 128 * (k + 1)],
                         idr[64:96, :], False, k == KT - 1)
            ins.then_inc(sems["s4"], 1)
        cls_chunks(NWAVE * NCH - cls_next[0])

    # ================= Act =================
    def act(e):
        e.wait_ge(sems["sdiB"], 16 * NDMA_B)
        cls_next = [0]

        def cls_copies(n):
            for _ in range(n):
                C = cls_next[0]
                v, ch = cls_wave_of(C)
                if v >= NWAVE:
                    return
                e.wait_ge(sems["s5"], C + 1)
                if v >= 2:
                    e.wait_ge(sems["sdo%d" % (v % 2)], 16 * (v // 2))
                ins = e.activation(obs[:, v % 2, VCH * ch : VCH * (ch + 1)],
                                   cps[C % 2][:, :], AF.Copy)
                ins.then_inc(sems["p1"], 1)
                cls_next[0] += 1

        for t in range(T):
            s0 = t % 2
            g1 = lambda a, b: G1[32 * s0 : 32 * s0 + 32, a:b]
            if t >= 6:
                cls_copies(2)
            e.wait_ge(sems["s1"], t + 1)
            ins = e.activation(sfio1[:, 0 : 2 * H], g1(0, 2 * H), AF.Sigmoid)
            ins.then_inc(sems["a1"], 1)
            ins = e.activation(tcc1[:], g1(3 * H, 4 * H), AF.Tanh)
            ins.then_inc(sems["a2"], 1)
            ins = e.activation(sfio1[:, 2 * H : 3 * H], g1(2 * H, 3 * H), AF.Sigmoid)
            ins.then_inc(sems["a3"], 1)
            e.wait_ge(sems["d1"], t + 1)
            ins = e.activation(tch1[:], c1[:], AF.Tanh)
            ins.then_inc(sems["a5"], 1)
            # L2 (rows 64-95 of G1)
            e.wait_ge(sems["s2"], t + 1)
            ins = e.activation(sfio2[64:96, 0 : 2 * H], G1[64:96, 0 : 2 * H],
                               AF.Sigmoid)
            ins.then_inc(sems["b1"], 1)
            ins = e.activation(tcc2[64:96, :], G1[64:96, 3 * H : 4 * H], AF.Tanh)
            ins.then_inc(sems["b2"], 1)
            ins = e.activation(sfio2[64:96, 2 * H : 3 * H],
                               G1[64:96, 2 * H : 3 * H], AF.Sigmoid)
            ins.then_inc(sems["b3"], 1)
            e.wait_ge(sems["d2"], t + 1)
            ins = e.activation(tch2[64:96, :], c2[64:96, :], AF.Tanh)
            ins.then_inc(sems["b5"], 1)
        cls_copies(NWAVE * NCH - cls_next[0])

    # ================= DVE =================
    def dve(e):
        e.wait_ge(sems["sdiB"], 16 * NDMA_B)
        for t in range(T):
            e.wait_ge(sems["a1"], t + 1)
            e.tensor_mul(out=tm1[:], in0=sfio1[:, 0:H], in1=c1[:])
            e.wait_ge(sems["a2"], t + 1)
            e.tensor_mul(out=tm2[:], in0=sfio1[:, H : 2 * H], in1=tcc1[:])
            e.drain()
            ins = e.tensor_add(out=c1[:], in0=tm1[:], in1=tm2[:])
            ins.then_inc(sems["d1"], 1)
            e.wait_ge(sems["a5"], t + 1)
            ins = e.tensor_mul(out=h1r[:], in0=sfio1[:, 2 * H : 3 * H], in1=tch1[:])
            ins.then_inc(sems["d4"], 1)
            e.wait_ge(sems["s3"], t + 1)
            ins = e.tensor_copy(out=hT1[:].rearrange("p k b -> p (k b)"),
                                in_=trp_h1_all)
            ins.then_inc(sems["d5"], 1)
            # L2 (rows 64-95)
            e.wait_ge(sems["b1"], t + 1)
            e.tensor_mul(out=tm3[64:96, :], in0=sfio2[64:96, 0:H],
                         in1=c2[64:96, :])
            e.wait_ge(sems["b2"], t + 1)
            e.tensor_mul(out=tm4[64:96, :], in0=sfio2[64:96, H : 2 * H],
                         in1=tcc2[64:96, :])
            e.drain()
            ins = e.tensor_add(out=c2[64:96, :], in0=tm3[64:96, :],
                               in1=tm4[64:96, :])
            ins.then_inc(sems["d2"], 1)
            e.wait_ge(sems["b5"], t + 1)
            ins = e.tensor_mul(out=h2r[64:96, :], in0=sfio2[64:96, 2 * H : 3 * H],
                               in1=tch2[64:96, :])
            ins.then_inc(sems["e4"], 1)
            e.wait_ge(sems["s4"], t + 1)
            cp2 = e.tensor_copy(out=hT2[:].rearrange("p k b -> p (k b)"),
                                in_=trp_h2_all)
            cp2.then_inc(sems["e5"], 1)
            if t >= 1:
                v, d = (t - 1) // 4, (t - 1) % 4
                if v >= 2:
                    e.wait_ge(sems["s5"], NCH * (v - 1))
                ins = e.tensor_copy(
                    out=ysT[:, :, v % 2, 32 * d : 32 * (d + 1)],
                    in_=trp_h2_all.rearrange("p (k b) -> p k b", b=B),
                )
                ins.then_inc(sems["e6"], 1)

    # ================= SP (HWDGE): init loads, gx->PSUM, even-wave stores ====
    def sp(e):
        for k in range(KT):
            e.dma_start(out=W1[:, k], in_=W1_d[k]).then_inc(sems["sdiB"], 16)
            e.dma_start(out=W2[:, k], in_=W2_d[k]).then_inc(sems["sdiB"], 16)
            e.dma_start(out=U2[:, k], in_=U2_d[k]).then_inc(sems["sdiB"], 16)
        for k in range(KT):
            e.dma_start(out=hT1[:, k, :], in_=h0_d[0, k]).then_inc(sems["sdiB"], 16)
            e.dma_start(out=hT2[:, k, :], in_=h0_d[1, k]).then_inc(sems["sdiB"], 16)
        e.dma_start(out=c1[:], in_=c0_d[0]).then_inc(sems["sdiB"], 16)
        e.dma_start(out=c2[64:96, :], in_=c0_d[1]).then_inc(sems["sdiB"], 16)
        e.dma_start(out=idr[:], in_=idr_d[:]).then_inc(sems["sdiB"], 16)
        for k in range(KT):
            e.dma_start(out=Wc[:, k], in_=Wc_d[k]).then_inc(sems["sdiC"], 16)
        # gx -> SBUF ring, interleaved with even-wave output stores
        ov = 0
        for w in range(NGXW):
            m0 = 128 * w
            mw = min(128, TB - m0)
            if w >= 2:
                e.wait_ge(sems["s1"], 4 * (w - 1))   # ring slot read out
            ins = e.dma_start(out=gxr[0:mw, w % 2, :], in_=gx_d[m0 : m0 + mw, :])
            ins.then_inc(sems["sgx%d" % (w % 2)], 16)
            while ov < NWAVE and ov + 3 <= w:
                e.wait_ge(sems["p1"], NCH * (ov + 1))
                ins = e.dma_start(
                    out=out_d[:, 4 * ov : 4 * ov + 4, :].rearrange("b s v -> s b v"),
                    in_=obs[:, ov % 2, :],
                )
                ins.then_inc(sems["sdo%d" % (ov % 2)], 16)
                ov += 2
        while ov < NWAVE:
            e.wait_ge(sems["p1"], NCH * (ov + 1))
            ins = e.dma_start(
                out=out_d[:, 4 * ov : 4 * ov + 4, :].rearrange("b s v -> s b v"),
                in_=obs[:, ov % 2, :],
            )
            ins.then_inc(sems["sdo%d" % (ov % 2)], 16)
            ov += 2

    # ================= Pool (SWDGE): odd-wave output stores =================
    def pool(e):
        for v in range(1, NWAVE, 2):
            e.wait_ge(sems["p1"], NCH * (v + 1))
            ins = e.dma_start(
                out=out_d[:, 4 * v : 4 * v + 4, :].rearrange("b s v -> s b v"),
                in_=obs[:, v % 2, :],
            )
            ins.then_inc(sems["sdo%d" % (v % 2)], 16)

    with nc.Block() as block:
        block.tensor(pe)
        block.scalar(act)
        block.vector(dve)
        block.gpsimd(pool)
        block.sync(sp)


def _prep(inputs):
    f = lambda k: np.asarray(inputs[k], np.float32)
    import ml_dtypes

    bf = ml_dtypes.bfloat16

    im_feat, embed = f("im_feat"), f("embed")
    W_im, b_im = f("W_im"), f("b_im")
    Wh, bw, Uh, bu = f("Wh"), f("bw"), f("Uh"), f("bu")
    Wxh, bxh, Wc, bc = f("Wxh"), f("bxh"), f("Wc"), f("bc")
    tokens = np.asarray(inputs["tokens"])
    h0, c0 = f("h0"), f("c0")

    zeros = all(not np.any(x) for x in (bw, bu, bxh, bc, b_im))

    y_im = im_feat @ W_im + b_im
    x_full = np.empty((T, B, D), np.float32)
    x_full[0] = y_im
    x_full[1:] = embed[tokens].transpose(1, 0, 2)

    U2p = (Wxh[0] @ Uh[1]).astype(np.float32)
    Wcy = (Wxh[1] @ Wc).astype(np.float32)
    gx = (x_full.reshape(TB, D) @ Uh[0]).astype(bf)           # [TB, 2048]

    ident = np.vstack([np.eye(B)] * 4).astype(bf)
    shared = {
        "gx": gx,
        "W1": np.ascontiguousarray(Wh[0].reshape(KT, 128, G4)).astype(bf),
        "W2": np.ascontiguousarray(Wh[1].reshape(KT, 128, G4)).astype(bf),
        "U2": np.ascontiguousarray(U2p.reshape(KT, 128, G4)).astype(bf),
        "h0T": np.ascontiguousarray(
            h0.transpose(0, 2, 1).reshape(L, KT, 128, B)
        ).astype(bf),
        "c0r": np.ascontiguousarray(c0),
        "idr": ident,
    }
    per_core = []
    for c in range(NCORES):
        vs = slice(VSH * c, VSH * (c + 1))
        m = dict(shared)
        m["Wcy"] = np.ascontiguousarray(Wcy[:, vs].reshape(KT, 128, VSH)).astype(bf)
        per_core.append(m)
    return per_core, zeros


def _numpy_ref(inputs):
    f = lambda k: np.asarray(inputs[k], np.float32)
    im_feat, embed = f("im_feat"), f("embed")
    Wh, bw, Uh, bu = f("Wh"), f("bw"), f("Uh"), f("bu")
    Wxh, bxh, Wc, bc = f("Wxh"), f("bxh"), f("Wc"), f("bc")
    tokens = np.asarray(inputs["tokens"])
    h = [f("h0")[l] for l in range(L)]
    c = [f("c0")[l] for l in range(L)]
    sig = lambda x: 1.0 / (1.0 + np.exp(-x))

    def step(hs, cs, xt):
        y = xt
        for l in range(L):
            gg = hs[l] @ Wh[l] + y @ Uh[l] + (bw[l] + bu[l])
            fg, ig, og, cc = np.split(gg, 4, axis=-1)
            cs[l] = sig(fg) * cs[l] + sig(ig) * np.tanh(cc)
            hs[l] = sig(og) * np.tanh(cs[l])
            y = hs[l] @ Wxh[l] + bxh[l]
        return y

    step(h, c, im_feat @ f("W_im") + f("b_im"))
    x_embed = embed[tokens]
    ys = np.stack([step(h, c, x_embed[:, t]) for t in range(S)], axis=1)
    return (ys @ Wc + bc).astype(np.float32)


def kernel(**inputs) -> np.ndarray:
    per_core, zeros = _prep(inputs)
    if not zeros:
        return _numpy_ref(inputs)

    from contextlib import ExitStack

    import concourse.bass as bass
    import concourse.mybir as mybir
    from concourse.bass_utils import run_bass_kernel_spmd

    nc = bass.Bass(target_bir_lowering=False)
    with ExitStack() as ctx:
        _build(nc, bass, mybir, ctx)

    core_ids = list(range(NCORES))
    res = run_bass_kernel_spmd(nc, per_core, core_ids, trace=TRACE)
    global _last_res
    _last_res = res
    return np.concatenate(
        [np.asarray(res.results[i]["out"]) for i in core_ids], axis=-1
    ).astype(np.float32)


_last_res = None


if __name__ == "__main__":
    sys.path.insert(0, "/root/problem")
    import reference

    ins = {k: np.asarray(v) for k, v in reference.setup_inputs().items()}
    out = kernel(**ins)
    print(out.shape, out.dtype)
